# revision 15
# baseline (speedup 1.0000x reference)
"""Trainium2 Bass kernel for NRI-style GNN decoder (nn_Decoder_58600533787128).

Data-parallel over batch across 8 NeuronCores.  All matmuls are bf16 with
free dim >= 512 (small-free matmuls measured ~10x slower per instruction
on HW): layer2 runs transposed with two edge types packed into the 128
partitions, the edge->node aggregation is a strided DVE reduction over the
receiver-major dense pair grid, and the output MLP is batched over all 8
per-core batches in one free-512 pass.

Pair grid: e = j*64 + i (receiver-major, 4096 pairs incl. diagonal;
diagonal killed by rel_type=0).  Supertile st covers receivers
[8st, 8st+8).  Per batch:
  pre^T[f,e]   = gather [x^T S; x^T R] via one-hot matmuls    (PE)
  h1^T[h,e]    = relu(W1^T @ pre^T + b1)  per type            (ACT/DVE)
  mT[o2,e]     = W2^T @ h1^T   2 types on partition halves    (PE)
  r2           = relu(mT + b2-packed)                         (ACT/DVE)
  s            = r2 * rt-packed (rank-1 selector matmul)      (DVE)
  red[o2,8]    = sum_i s[o2, j*64+i]  contiguous reduce       (DVE)
  aggT2[o2,j] += red   (types 0+2 on top half, 1+3 bottom)    (Pool)
  augA[.,b]    = [x^T ; agg-bottom], aggA_top separate        (Pool)
Then one batched MLP over augA [128, 512] with an extra accumulating
matmul folding aggA_top in through ow1's agg rows.
"""
import sys

sys.path.insert(0, "/opt/trn_rl_repo")

import numpy as np
import ml_dtypes

BF16 = ml_dtypes.bfloat16

B, N, F, H, O, T, E = 64, 64, 64, 256, 64, 4, 4032
EP = N * N         # dense pair grid (j,i), 4096, includes diagonal
NST = 8            # supertiles of 512 pairs (8 receivers each)
NCORES = 8
BPC = B // NCORES  # batches per core


def build_nc(bpc=BPC, num_devices=NCORES, reps=1):
    import concourse.mybir as mybir
    from concourse import bacc, tile

    f32 = mybir.dt.float32
    bf16 = mybir.dt.bfloat16
    AF = mybir.ActivationFunctionType
    ALU = mybir.AluOpType
    AX = mybir.AxisListType

    nc = bacc.Bacc(
        "TRN2", target_bir_lowering=False, debug=False, num_devices=num_devices
    )
    x_d = nc.declare_dram_parameter("x", [bpc, N, F], bf16, isOutput=False)
    xT_d = nc.declare_dram_parameter("xT", [bpc, F, N], bf16, isOutput=False)
    rt_d = nc.declare_dram_parameter("rt32", [bpc, T * NST, 512], bf16, isOutput=False)
    rsT_d = nc.declare_dram_parameter("rsT", [N, EP], bf16, isOutput=False)
    rrT_d = nc.declare_dram_parameter("rrT", [N, EP], bf16, isOutput=False)
    sel2_d = nc.declare_dram_parameter(
        "sel2", [T * NST, 2 * NST * 128], bf16, isOutput=False)
    w1_d = nc.declare_dram_parameter("w1s", [128, T * H], bf16, isOutput=False)
    b1_d = nc.declare_dram_parameter("b1c", [128, T * 2], f32, isOutput=False)
    w2_d = nc.declare_dram_parameter("w2s", [128, T * 2 * O], bf16, isOutput=False)
    b2_d = nc.declare_dram_parameter("b2p", [128, 2], f32, isOutput=False)
    ow1_d = nc.declare_dram_parameter("ow1s", [128, H], bf16, isOutput=False)
    ow1b_d = nc.declare_dram_parameter("ow1b0", [N, H], bf16, isOutput=False)
    ob1_d = nc.declare_dram_parameter("ob1c", [128, 2], f32, isOutput=False)
    ow2_d = nc.declare_dram_parameter("ow2s", [128, 2 * H], bf16, isOutput=False)
    ob2_d = nc.declare_dram_parameter("ob2c", [128, 2], f32, isOutput=False)
    ow3_d = nc.declare_dram_parameter("ow3s", [128, 2 * O], bf16, isOutput=False)
    ob3_d = nc.declare_dram_parameter("ob3c", [O, 1], f32, isOutput=False)
    y_d = nc.declare_dram_parameter("y", [O, bpc * N], f32, isOutput=True)

    with tile.TileContext(nc) as tc:
        with (
            tc.tile_pool(name="const", bufs=1) as cpool,
            tc.tile_pool(name="work", bufs=3) as wpool,
            tc.tile_pool(name="h1pool", bufs=4) as hpool,
            tc.tile_pool(name="spool", bufs=4) as spool,
            tc.tile_pool(name="ppre", bufs=1, space="PSUM") as ppre,
            tc.tile_pool(name="ph1", bufs=3, space="PSUM") as ph1,
            tc.tile_pool(name="pmT", bufs=2, space="PSUM") as pmT,
            tc.tile_pool(name="prt", bufs=2, space="PSUM") as prt,
        ):
            # resident constants (one DMA each; layouts prepped host-side)
            rsT = cpool.tile([N, EP], bf16)
            nc.sync.dma_start(rsT[:], rsT_d[:])
            rrT = cpool.tile([N, EP], bf16)
            nc.sync.dma_start(rrT[:], rrT_d[:])
            sel2 = cpool.tile([T * NST, 2 * NST * 128], bf16)
            nc.sync.dma_start(sel2[:], sel2_d[:])
            w1s = cpool.tile([128, T * H], bf16)
            nc.sync.dma_start(w1s[:], w1_d[:])
            b1c = cpool.tile([128, T * 2], f32)
            nc.sync.dma_start(b1c[:], b1_d[:])
            w2s = cpool.tile([128, T * 2 * O], bf16)
            nc.sync.dma_start(w2s[:], w2_d[:])
            b2p = cpool.tile([128, 2], f32)
            nc.sync.dma_start(b2p[:], b2_d[:])
            ow1s = cpool.tile([128, H], bf16)
            nc.sync.dma_start(ow1s[:], ow1_d[:])
            ow1b0 = cpool.tile([N, H], bf16)
            nc.sync.dma_start(ow1b0[:], ow1b_d[:])
            ob1c = cpool.tile([128, 2], f32)
            nc.sync.dma_start(ob1c[:], ob1_d[:])
            ow2s = cpool.tile([128, 2 * H], bf16)
            nc.sync.dma_start(ow2s[:], ow2_d[:])
            ob2c = cpool.tile([128, 2], f32)
            nc.sync.dma_start(ob2c[:], ob2_d[:])
            ow3s = cpool.tile([128, 2 * O], bf16)
            nc.sync.dma_start(ow3s[:], ow3_d[:])
            ob3c = cpool.tile([O, 1], f32)
            nc.sync.dma_start(ob3c[:], ob3_d[:])

            # ACT / DVE balancer for PSUM-reading element ops
            busy = [0.0, 0.0]

            def pick(costs):
                e = min(range(2), key=lambda i: busy[i] + costs[i])
                busy[e] += costs[e]
                return e

            def relu_bias(dst, src, bias_col, cols):
                e = pick([cols / 1.2 + 180, cols / 0.96 + 125])
                if e == 0:
                    nc.scalar.activation(dst, src, AF.Relu, bias=bias_col)
                else:
                    nc.vector.tensor_scalar(dst, src, bias_col, 0.0, ALU.add, ALU.max)

            def copy_op(dst, src, cols):
                e = pick([cols / 1.2 + 180, cols / 0.96 + 125])
                if e == 0:
                    nc.scalar.activation(dst, src, AF.Copy)
                else:
                    nc.vector.tensor_copy(dst, src)

            import contextlib
            loop_cm = tc.For_i(0, reps, 1) if reps > 1 else contextlib.nullcontext()
            with loop_cm:
              augA = wpool.tile([128, bpc * N], bf16, tag="augA")
              aggTopA = wpool.tile([N, bpc * N], bf16, tag="aggTopA")
              for b in range(bpc):
                x_sb = wpool.tile([N, F], bf16, tag="x_sb")
                nc.sync.dma_start(x_sb[:], x_d[b])
                nc.sync.dma_start(augA[0:F, b * N:(b + 1) * N], xT_d[b])
                rt32 = wpool.tile([T * NST, 512], bf16, tag="rt32")
                nc.sync.dma_start(rt32[:], rt_d[b])

                aggT2 = wpool.tile([128, N], f32, tag="aggT2")

                # tail of a pipeline slot: layer2 + relu2 + rt scale + reduce
                # + aggregate, emitted one (st,tp) slot late so the in-order
                # PE queue always has independent work (h1 matmuls of the
                # next slot) while ACT/DVE produce this slot's h1s
                def emit_tail(slot):
                    st, tp, h1pair, rtp = slot
                    mTp = pmT.tile([128, 512], f32, tag="mTp")
                    for ti in range(2):
                        t = 2 * tp + ti
                        for kc in range(2):
                            nc.tensor.matmul(
                                mTp[ti * 64:(ti + 1) * 64, :],
                                w2s[:, (t * 2 + kc) * O:(t * 2 + kc + 1) * O],
                                h1pair[ti][:, kc * 512:(kc + 1) * 512],
                                start=(kc == 0), stop=(kc == 1),
                                skip_group_check=True,
                            )
                    r2 = spool.tile([128, 512], bf16, tag="r2")
                    relu_bias(r2[:], mTp[:], b2p[:, tp:tp + 1], 512)
                    s = spool.tile([128, 512], bf16, tag="s")
                    nc.vector.tensor_tensor(s[:], r2[:], rtp[:], ALU.mult)
                    red = spool.tile([128, NST], f32, tag="red")
                    nc.vector.tensor_reduce(
                        red[:],
                        s[:].rearrange("p (j i) -> p j i", j=NST, i=N),
                        AX.X, ALU.add,
                    )
                    if tp == 0:
                        nc.gpsimd.tensor_copy(
                            aggT2[:, st * NST:(st + 1) * NST], red[:])
                    else:
                        nc.gpsimd.tensor_tensor(
                            aggT2[:, st * NST:(st + 1) * NST],
                            aggT2[:, st * NST:(st + 1) * NST], red[:],
                            ALU.add,
                        )

                pend = None
                for st in range(NST):
                    e0 = st * 512
                    # gather: pre^T = [senders^T ; receivers^T] for 512 pairs
                    prep = ppre.tile([128, 512], f32, tag="prep")
                    nc.tensor.matmul(
                        prep[0:64, :], x_sb[:], rsT[:, e0:e0 + 512],
                        start=True, stop=True,
                    )
                    nc.tensor.matmul(
                        prep[64:128, :], x_sb[:], rrT[:, e0:e0 + 512],
                        start=True, stop=True,
                    )
                    preT = wpool.tile([128, 512], bf16, tag="preT")
                    copy_op(preT[:], prep[:], 512)

                    for tp in range(2):
                        # rt rows for both types via one selector matmul
                        rtp = prt.tile([128, 512], f32, tag="rtp")
                        nc.tensor.matmul(
                            rtp[:],
                            sel2[:, (tp * NST + st) * 128:(tp * NST + st + 1) * 128],
                            rt32[:],
                            start=True, stop=True,
                        )
                        h1pair = []
                        for ti in range(2):
                            t = 2 * tp + ti
                            h1s = hpool.tile([128, 2 * 512], bf16, tag="h1s")
                            for hc in range(2):
                                h1p = ph1.tile([128, 512], f32, tag="h1p")
                                nc.tensor.matmul(
                                    h1p[:],
                                    w1s[:, t * H + hc * 128: t * H + (hc + 1) * 128],
                                    preT[:],
                                    start=True, stop=True,
                                )
                                relu_bias(
                                    h1s[:, hc * 512:(hc + 1) * 512], h1p[:],
                                    b1c[:, t * 2 + hc: t * 2 + hc + 1], 512,
                                )
                            h1pair.append(h1s)
                            if ti == 0 and pend is not None:
                                # previous slot's tail between the two h1
                                # tiles: its layer2 matmuls cover the PE while
                                # ACT/DVE drain this slot's first h1 banks
                                emit_tail(pend)
                                pend = None
                        if pend is not None:
                            emit_tail(pend)
                        pend = (st, tp, h1pair, rtp)
                emit_tail(pend)
                pend = None

                # agg split: types 0+2 on partitions 0..63 -> aggTopA (base 0),
                # types 1+3 on partitions 64..127 -> augA agg half (lane-aligned)
                nc.gpsimd.tensor_copy(
                    aggTopA[:, b * N:(b + 1) * N], aggT2[0:N, :])
                nc.gpsimd.tensor_copy(
                    augA[F:128, b * N:(b + 1) * N], aggT2[N:128, :])

              # batched output MLP over all bpc batches (free = bpc*N = 512)
              W = bpc * N
              f1 = wpool.tile([128, 2 * W], bf16, tag="f1")
              for mc in range(2):
                  fp = pmT.tile([128, 512], f32, tag="mTp")
                  nc.tensor.matmul(
                      fp[:, 0:W], ow1s[:, mc * 128:(mc + 1) * 128], augA[:],
                      start=True, stop=False, skip_group_check=True,
                  )
                  nc.tensor.matmul(
                      fp[:, 0:W], ow1b0[:, mc * 128:(mc + 1) * 128], aggTopA[:],
                      start=False, stop=True, skip_group_check=True,
                  )
                  nc.scalar.activation(
                      f1[:, mc * W:(mc + 1) * W], fp[:, 0:W], AF.Relu,
                      bias=ob1c[:, mc:mc + 1],
                  )
              f2 = wpool.tile([128, 2 * W], bf16, tag="f2")
              for mc in range(2):
                  fp = pmT.tile([128, 512], f32, tag="mTp")
                  for kc in range(2):
                      nc.tensor.matmul(
                          fp[:, 0:W],
                          ow2s[:, kc * H + mc * 128: kc * H + (mc + 1) * 128],
                          f1[:, kc * W:(kc + 1) * W],
                          start=(kc == 0), stop=(kc == 1),
                      )
                  nc.scalar.activation(
                      f2[:, mc * W:(mc + 1) * W], fp[:, 0:W], AF.Relu,
                      bias=ob2c[:, mc:mc + 1],
                  )
              op = pmT.tile([128, 512], f32, tag="mTp")
              for kc in range(2):
                  nc.tensor.matmul(
                      op[0:O, 0:W], ow3s[:, kc * O:(kc + 1) * O],
                      f2[:, kc * W:(kc + 1) * W],
                      start=(kc == 0), stop=(kc == 1),
                  )
              yb = wpool.tile([O, W], f32, tag="yb")
              nc.vector.tensor_scalar(
                  yb[:], op[0:O, 0:W], ob3c[:, 0:1], None, ALU.add
              )
              nc.sync.dma_start(y_d[:], yb[:])

    nc.compile()
    return nc


def edge_maps(rel_rec, rel_send):
    """Pair-grid index (receiver-major) for each of the E directed edges."""
    send_idx = np.argmax(rel_send, axis=1).astype(np.int64)  # [E]
    rec_idx = np.argmax(rel_rec, axis=1).astype(np.int64)    # [E]
    return rec_idx * N + send_idx, rec_idx


def prep_shared(w1, b1, w2, b2, ow1, ob1, ow2, ob2, ow3, ob3):
    """Host-side layout prep for the replicated weights (bf16) + one-hots."""
    j_of = np.repeat(np.arange(N), N)         # receiver of pair e = j*64+i
    i_of = np.tile(np.arange(N), N)           # sender
    rsT = np.zeros((N, EP), np.float32)
    rsT[i_of, np.arange(EP)] = 1.0            # senders^T one-hot
    rrT = np.zeros((N, EP), np.float32)
    rrT[j_of, np.arange(EP)] = 1.0            # receivers^T one-hot
    R = T * NST
    sel2 = np.zeros((R, 2 * NST * 128), np.float32)
    for tp in range(2):
        for st in range(NST):
            base = (tp * NST + st) * 128
            sel2[2 * tp * NST + st, base:base + 64] = 1.0
            sel2[(2 * tp + 1) * NST + st, base + 64:base + 128] = 1.0
    w1s = np.ascontiguousarray(
        w1.transpose(1, 0, 2).reshape(2 * F, T * H)).astype(BF16)
    b1c = np.ascontiguousarray(
        b1.reshape(T, 2, 128).transpose(2, 0, 1).reshape(128, T * 2)
    ).astype(np.float32)
    w2s = np.ascontiguousarray(
        w2.reshape(T, 2, 128, O).transpose(2, 0, 1, 3).reshape(128, T * 2 * O)
    ).astype(BF16)
    b2pk = np.zeros((128, 2), np.float32)
    for tp in range(2):
        b2pk[0:64, tp] = b2[2 * tp]
        b2pk[64:128, tp] = b2[2 * tp + 1]
    ow1s = np.ascontiguousarray(ow1).astype(BF16)              # [128, H]
    ow1b0 = np.ascontiguousarray(ow1[N:2 * N]).astype(BF16)    # agg rows, base 0
    ob1c = np.ascontiguousarray(ob1.reshape(2, 128).T).astype(np.float32)
    ow2s = np.ascontiguousarray(
        ow2.reshape(2, 128, H).transpose(1, 0, 2).reshape(128, 2 * H)).astype(BF16)
    ob2c = np.ascontiguousarray(ob2.reshape(2, 128).T).astype(np.float32)
    ow3s = np.ascontiguousarray(
        ow3.reshape(2, 128, O).transpose(1, 0, 2).reshape(128, 2 * O)).astype(BF16)
    ob3c = np.ascontiguousarray(ob3.reshape(O, 1)).astype(np.float32)
    return dict(
        rsT=rsT.astype(BF16), rrT=rrT.astype(BF16), sel2=sel2.astype(BF16),
        w1s=w1s, b1c=b1c, w2s=w2s, b2p=b2pk,
        ow1s=ow1s, ow1b0=ow1b0, ob1c=ob1c, ow2s=ow2s, ob2c=ob2c,
        ow3s=ow3s, ob3c=ob3c,
    )


def prep_batch(x, rel_type, e_of_edge, rec_idx):
    """Per-core batched tensors: x, xT, rel_type as [T*NST, 512] rows."""
    bpc = x.shape[0]
    xT = np.ascontiguousarray(x.transpose(0, 2, 1))            # [bpc, F, N]
    rt_pad = np.zeros((bpc, EP, T), np.float32)
    rt_pad[:, e_of_edge, :] = rel_type                          # diag stays 0
    # row t*NST+st holds rel_type for pairs [st*512, (st+1)*512), type t
    rt32 = np.ascontiguousarray(
        rt_pad.reshape(bpc, NST, 512, T).transpose(0, 3, 1, 2).reshape(
            bpc, T * NST, 512)
    ).astype(BF16)
    return dict(x=np.ascontiguousarray(x).astype(BF16), xT=xT.astype(BF16),
                rt32=rt32)


def kernel(**inputs):
    from concourse.bass_utils import run_bass_kernel_spmd

    f32arrs = {k: np.asarray(v, dtype=np.float32) for k, v in inputs.items()}
    shared = prep_shared(
        f32arrs["w1"], f32arrs["b1"], f32arrs["w2"], f32arrs["b2"],
        f32arrs["ow1"], f32arrs["ob1"], f32arrs["ow2"], f32arrs["ob2"],
        f32arrs["ow3"], f32arrs["ob3"],
    )
    e_of_edge, rec_idx = edge_maps(f32arrs["rel_rec"], f32arrs["rel_send"])
    in_maps = []
    for c in range(NCORES):
        sl = slice(c * BPC, (c + 1) * BPC)
        m = dict(shared)
        m.update(prep_batch(
            f32arrs["x"][sl], f32arrs["rel_type"][sl], e_of_edge, rec_idx))
        in_maps.append(m)

    nc = build_nc(BPC)
    res = run_bass_kernel_spmd(nc, in_maps, list(range(NCORES)))
    # y per core: [O, BPC*N] -> [BPC, N, O]; concat -> full [B, N, O]
    y = np.concatenate(
        [res.results[c]["y"].reshape(O, BPC, N).transpose(1, 2, 0)
         for c in range(NCORES)], axis=0)
    return np.ascontiguousarray(y).astype(np.float32)


if __name__ == "__main__":
    # smoke: random inputs, shape check only
    rng = np.random.default_rng(0)
    eye = np.eye(N, dtype=np.float32)
    si, ri = [], []
    for i in range(N):
        for j in range(N):
            if i != j:
                si.append(i)
                ri.append(j)
    inputs = {
        "x": rng.standard_normal((B, N, F), dtype=np.float32),
        "rel_type": rng.random((B, E, T), dtype=np.float32),
        "rel_rec": eye[np.array(ri)],
        "rel_send": eye[np.array(si)],
        "w1": rng.standard_normal((T, 2 * F, H), dtype=np.float32) * 0.1,
        "b1": rng.standard_normal((T, H), dtype=np.float32) * 0.1,
        "w2": rng.standard_normal((T, H, O), dtype=np.float32) * 0.1,
        "b2": rng.standard_normal((T, O), dtype=np.float32) * 0.1,
        "ow1": rng.standard_normal((F + O, H), dtype=np.float32) * 0.1,
        "ob1": rng.standard_normal((H,), dtype=np.float32) * 0.1,
        "ow2": rng.standard_normal((H, H), dtype=np.float32) * 0.1,
        "ob2": rng.standard_normal((H,), dtype=np.float32) * 0.1,
        "ow3": rng.standard_normal((H, O), dtype=np.float32) * 0.1,
        "ob3": rng.standard_normal((O,), dtype=np.float32) * 0.1,
    }
    y = kernel(**inputs)
    print("y", y.shape, y.dtype)


# revision 17
# speedup vs baseline: 1.4752x; 1.4752x over previous
"""Trainium2 Bass kernel for NRI-style GNN decoder (nn_Decoder_58600533787128).

Data-parallel over batch across 8 NeuronCores.  All matmuls are bf16 with
free dim >= 512 (small-free matmuls measured ~10x slower per instruction
on HW): layer2 runs transposed with two edge types packed into the 128
partitions, the edge->node aggregation is a strided DVE reduction over the
receiver-major dense pair grid, and the output MLP is batched over all 8
per-core batches in one free-512 pass.

Pair grid: e = j*64 + i (receiver-major, 4096 pairs incl. diagonal;
diagonal killed by rel_type=0).  Supertile st covers receivers
[8st, 8st+8).  Per batch:
  pre^T[f,e]   = gather [x^T S; x^T R] via one-hot matmuls    (PE)
  h1^T[h,e]    = relu(W1^T @ pre^T + b1)  per type            (ACT/DVE)
  mT[o2,e]     = W2^T @ h1^T   2 types on partition halves    (PE)
  r2           = relu(mT + b2-packed)                         (ACT/DVE)
  s            = r2 * rt-packed (rank-1 selector matmul)      (DVE)
  red[o2,8]    = sum_i s[o2, j*64+i]  contiguous reduce       (DVE)
  aggT2[o2,j] += red   (types 0+2 on top half, 1+3 bottom)    (Pool)
  augA[.,b]    = [x^T ; agg-bottom], aggA_top separate        (Pool)
Then one batched MLP over augA [128, 512] with an extra accumulating
matmul folding aggA_top in through ow1's agg rows.
"""
import sys

sys.path.insert(0, "/opt/trn_rl_repo")

import numpy as np
import ml_dtypes

BF16 = ml_dtypes.bfloat16

B, N, F, H, O, T, E = 64, 64, 64, 256, 64, 4, 4032
EP = N * N         # dense pair grid (j,i), 4096, includes diagonal
NST = 8            # supertiles of 512 pairs (8 receivers each)
NCORES = 8
BPC = B // NCORES  # batches per core


def build_nc(bpc=BPC, num_devices=NCORES, reps=1):
    import concourse.mybir as mybir
    from concourse import bacc, tile

    f32 = mybir.dt.float32
    bf16 = mybir.dt.bfloat16
    AF = mybir.ActivationFunctionType
    ALU = mybir.AluOpType
    AX = mybir.AxisListType

    nc = bacc.Bacc(
        "TRN2", target_bir_lowering=False, debug=False, num_devices=num_devices
    )
    x_d = nc.declare_dram_parameter("x", [bpc, N, F], bf16, isOutput=False)
    xT_d = nc.declare_dram_parameter("xT", [bpc, F, N], bf16, isOutput=False)
    rt_d = nc.declare_dram_parameter("rt32", [bpc, T * NST, 512], bf16, isOutput=False)
    rsT_d = nc.declare_dram_parameter("rsT", [N, EP], bf16, isOutput=False)
    rrT_d = nc.declare_dram_parameter("rrT", [N, EP], bf16, isOutput=False)
    sel2_d = nc.declare_dram_parameter(
        "sel2", [T * NST, 2 * NST * 128], bf16, isOutput=False)
    w1_d = nc.declare_dram_parameter("w1s", [128, T * H], bf16, isOutput=False)
    b1_d = nc.declare_dram_parameter("b1c", [128, T * 2], f32, isOutput=False)
    w2_d = nc.declare_dram_parameter("w2s", [128, T * 2 * O], bf16, isOutput=False)
    b2_d = nc.declare_dram_parameter("b2p", [128, 2], f32, isOutput=False)
    ow1_d = nc.declare_dram_parameter("ow1s", [128, H], bf16, isOutput=False)
    ow1b_d = nc.declare_dram_parameter("ow1b0", [N, H], bf16, isOutput=False)
    ob1_d = nc.declare_dram_parameter("ob1c", [128, 2], f32, isOutput=False)
    ow2_d = nc.declare_dram_parameter("ow2s", [128, 2 * H], bf16, isOutput=False)
    ob2_d = nc.declare_dram_parameter("ob2c", [128, 2], f32, isOutput=False)
    ow3_d = nc.declare_dram_parameter("ow3s", [128, 2 * O], bf16, isOutput=False)
    ob3_d = nc.declare_dram_parameter("ob3c", [O, 1], f32, isOutput=False)
    y_d = nc.declare_dram_parameter("y", [O, bpc * N], f32, isOutput=True)

    with tile.TileContext(nc) as tc:
        with (
            tc.tile_pool(name="const", bufs=1) as cpool,
            tc.tile_pool(name="work", bufs=4) as wpool,
            tc.tile_pool(name="h1pool", bufs=6) as hpool,
            tc.tile_pool(name="spool", bufs=6) as spool,
            tc.tile_pool(name="ppre", bufs=1, space="PSUM") as ppre,
            tc.tile_pool(name="ph1", bufs=3, space="PSUM") as ph1,
            tc.tile_pool(name="pmT", bufs=2, space="PSUM") as pmT,
            tc.tile_pool(name="prt", bufs=2, space="PSUM") as prt,
        ):
            # resident constants (one DMA each; layouts prepped host-side)
            rsT = cpool.tile([N, EP], bf16)
            nc.sync.dma_start(rsT[:], rsT_d[:])
            rrT = cpool.tile([N, EP], bf16)
            nc.sync.dma_start(rrT[:], rrT_d[:])
            sel2 = cpool.tile([T * NST, 2 * NST * 128], bf16)
            nc.sync.dma_start(sel2[:], sel2_d[:])
            w1s = cpool.tile([128, T * H], bf16)
            nc.sync.dma_start(w1s[:], w1_d[:])
            b1c = cpool.tile([128, T * 2], f32)
            nc.sync.dma_start(b1c[:], b1_d[:])
            w2s = cpool.tile([128, T * 2 * O], bf16)
            nc.sync.dma_start(w2s[:], w2_d[:])
            b2p = cpool.tile([128, 2], f32)
            nc.sync.dma_start(b2p[:], b2_d[:])
            ow1s = cpool.tile([128, H], bf16)
            nc.sync.dma_start(ow1s[:], ow1_d[:])
            ow1b0 = cpool.tile([N, H], bf16)
            nc.sync.dma_start(ow1b0[:], ow1b_d[:])
            ob1c = cpool.tile([128, 2], f32)
            nc.sync.dma_start(ob1c[:], ob1_d[:])
            ow2s = cpool.tile([128, 2 * H], bf16)
            nc.sync.dma_start(ow2s[:], ow2_d[:])
            ob2c = cpool.tile([128, 2], f32)
            nc.sync.dma_start(ob2c[:], ob2_d[:])
            ow3s = cpool.tile([128, 2 * O], bf16)
            nc.sync.dma_start(ow3s[:], ow3_d[:])
            ob3c = cpool.tile([O, 1], f32)
            nc.sync.dma_start(ob3c[:], ob3_d[:])

            # ACT / DVE balancer for PSUM-reading element ops
            busy = [0.0, 0.0]

            def pick(costs):
                e = min(range(2), key=lambda i: busy[i] + costs[i])
                busy[e] += costs[e]
                return e

            def relu_bias(dst, src, bias_col, cols):
                e = pick([cols / 1.2 + 180, cols / 0.96 + 125])
                if e == 0:
                    nc.scalar.activation(dst, src, AF.Relu, bias=bias_col)
                else:
                    nc.vector.tensor_scalar(dst, src, bias_col, 0.0, ALU.add, ALU.max)

            def copy_op(dst, src, cols):
                e = pick([cols / 1.2 + 180, cols / 0.96 + 125])
                if e == 0:
                    nc.scalar.activation(dst, src, AF.Copy)
                else:
                    nc.vector.tensor_copy(dst, src)

            import contextlib
            loop_cm = tc.For_i(0, reps, 1) if reps > 1 else contextlib.nullcontext()
            with loop_cm:
              augA = wpool.tile([128, bpc * N], bf16, tag="augA")
              aggTopA = wpool.tile([N, bpc * N], bf16, tag="aggTopA")
              for b in range(bpc):
                x_sb = wpool.tile([N, F], bf16, tag="x_sb")
                nc.sync.dma_start(x_sb[:], x_d[b])
                nc.sync.dma_start(augA[0:F, b * N:(b + 1) * N], xT_d[b])
                rt32 = wpool.tile([T * NST, 512], bf16, tag="rt32")
                nc.sync.dma_start(rt32[:], rt_d[b])

                aggT2 = wpool.tile([128, N], f32, tag="aggT2")

                # tail of a pipeline slot: layer2 + relu2 + rt scale + reduce
                # + aggregate, emitted one (st,tp) slot late so the in-order
                # PE queue always has independent work (h1 matmuls of the
                # next slot) while ACT/DVE produce this slot's h1s
                def emit_tail(slot):
                    st, tp, h1pair, rtp = slot
                    mTp = pmT.tile([128, 512], f32, tag="mTp")
                    for ti in range(2):
                        t = 2 * tp + ti
                        for kc in range(2):
                            nc.tensor.matmul(
                                mTp[ti * 64:(ti + 1) * 64, :],
                                w2s[:, (t * 2 + kc) * O:(t * 2 + kc + 1) * O],
                                h1pair[ti][:, kc * 512:(kc + 1) * 512],
                                start=(kc == 0), stop=(kc == 1),
                                skip_group_check=True,
                            )
                    r2 = spool.tile([128, 512], bf16, tag="r2")
                    relu_bias(r2[:], mTp[:], b2p[:, tp:tp + 1], 512)
                    s = spool.tile([128, 512], bf16, tag="s")
                    nc.vector.tensor_tensor(s[:], r2[:], rtp[:], ALU.mult)
                    red = spool.tile([128, NST], f32, tag="red")
                    nc.vector.tensor_reduce(
                        red[:],
                        s[:].rearrange("p (j i) -> p j i", j=NST, i=N),
                        AX.X, ALU.add,
                    )
                    if tp == 0:
                        nc.gpsimd.tensor_copy(
                            aggT2[:, st * NST:(st + 1) * NST], red[:])
                    else:
                        nc.gpsimd.tensor_tensor(
                            aggT2[:, st * NST:(st + 1) * NST],
                            aggT2[:, st * NST:(st + 1) * NST], red[:],
                            ALU.add,
                        )

                pend = None
                for st in range(NST):
                    e0 = st * 512
                    # gather: pre^T = [senders^T ; receivers^T] for 512 pairs
                    prep = ppre.tile([128, 512], f32, tag="prep")
                    nc.tensor.matmul(
                        prep[0:64, :], x_sb[:], rsT[:, e0:e0 + 512],
                        start=True, stop=True,
                    )
                    nc.tensor.matmul(
                        prep[64:128, :], x_sb[:], rrT[:, e0:e0 + 512],
                        start=True, stop=True,
                    )
                    preT = wpool.tile([128, 512], bf16, tag="preT")
                    copy_op(preT[:], prep[:], 512)

                    for tp in range(2):
                        # rt rows for both types via one selector matmul
                        rtp = prt.tile([128, 512], f32, tag="rtp")
                        nc.tensor.matmul(
                            rtp[:],
                            sel2[:, (tp * NST + st) * 128:(tp * NST + st + 1) * 128],
                            rt32[:],
                            start=True, stop=True,
                        )
                        h1pair = []
                        for ti in range(2):
                            t = 2 * tp + ti
                            h1s = hpool.tile([128, 2 * 512], bf16, tag="h1s")
                            for hc in range(2):
                                h1p = ph1.tile([128, 512], f32, tag="h1p")
                                nc.tensor.matmul(
                                    h1p[:],
                                    w1s[:, t * H + hc * 128: t * H + (hc + 1) * 128],
                                    preT[:],
                                    start=True, stop=True,
                                )
                                relu_bias(
                                    h1s[:, hc * 512:(hc + 1) * 512], h1p[:],
                                    b1c[:, t * 2 + hc: t * 2 + hc + 1], 512,
                                )
                            h1pair.append(h1s)
                        if pend is not None:
                            emit_tail(pend)
                        pend = (st, tp, h1pair, rtp)
                emit_tail(pend)
                pend = None

                # agg split: types 0+2 on partitions 0..63 -> aggTopA (base 0),
                # types 1+3 on partitions 64..127 -> augA agg half (lane-aligned)
                nc.gpsimd.tensor_copy(
                    aggTopA[:, b * N:(b + 1) * N], aggT2[0:N, :])
                nc.gpsimd.tensor_copy(
                    augA[F:128, b * N:(b + 1) * N], aggT2[N:128, :])

              # batched output MLP over all bpc batches (free = bpc*N = 512)
              W = bpc * N
              f1 = wpool.tile([128, 2 * W], bf16, tag="f1")
              for mc in range(2):
                  fp = pmT.tile([128, 512], f32, tag="mTp")
                  nc.tensor.matmul(
                      fp[:, 0:W], ow1s[:, mc * 128:(mc + 1) * 128], augA[:],
                      start=True, stop=False, skip_group_check=True,
                  )
                  nc.tensor.matmul(
                      fp[:, 0:W], ow1b0[:, mc * 128:(mc + 1) * 128], aggTopA[:],
                      start=False, stop=True, skip_group_check=True,
                  )
                  nc.scalar.activation(
                      f1[:, mc * W:(mc + 1) * W], fp[:, 0:W], AF.Relu,
                      bias=ob1c[:, mc:mc + 1],
                  )
              f2 = wpool.tile([128, 2 * W], bf16, tag="f2")
              for mc in range(2):
                  fp = pmT.tile([128, 512], f32, tag="mTp")
                  for kc in range(2):
                      nc.tensor.matmul(
                          fp[:, 0:W],
                          ow2s[:, kc * H + mc * 128: kc * H + (mc + 1) * 128],
                          f1[:, kc * W:(kc + 1) * W],
                          start=(kc == 0), stop=(kc == 1),
                      )
                  nc.scalar.activation(
                      f2[:, mc * W:(mc + 1) * W], fp[:, 0:W], AF.Relu,
                      bias=ob2c[:, mc:mc + 1],
                  )
              op = pmT.tile([128, 512], f32, tag="mTp")
              for kc in range(2):
                  nc.tensor.matmul(
                      op[0:O, 0:W], ow3s[:, kc * O:(kc + 1) * O],
                      f2[:, kc * W:(kc + 1) * W],
                      start=(kc == 0), stop=(kc == 1),
                  )
              yb = wpool.tile([O, W], f32, tag="yb")
              nc.vector.tensor_scalar(
                  yb[:], op[0:O, 0:W], ob3c[:, 0:1], None, ALU.add
              )
              nc.sync.dma_start(y_d[:], yb[:])

    nc.compile()
    return nc


def edge_maps(rel_rec, rel_send):
    """Pair-grid index (receiver-major) for each of the E directed edges."""
    send_idx = np.argmax(rel_send, axis=1).astype(np.int64)  # [E]
    rec_idx = np.argmax(rel_rec, axis=1).astype(np.int64)    # [E]
    return rec_idx * N + send_idx, rec_idx


def prep_shared(w1, b1, w2, b2, ow1, ob1, ow2, ob2, ow3, ob3):
    """Host-side layout prep for the replicated weights (bf16) + one-hots."""
    j_of = np.repeat(np.arange(N), N)         # receiver of pair e = j*64+i
    i_of = np.tile(np.arange(N), N)           # sender
    rsT = np.zeros((N, EP), np.float32)
    rsT[i_of, np.arange(EP)] = 1.0            # senders^T one-hot
    rrT = np.zeros((N, EP), np.float32)
    rrT[j_of, np.arange(EP)] = 1.0            # receivers^T one-hot
    R = T * NST
    sel2 = np.zeros((R, 2 * NST * 128), np.float32)
    for tp in range(2):
        for st in range(NST):
            base = (tp * NST + st) * 128
            sel2[2 * tp * NST + st, base:base + 64] = 1.0
            sel2[(2 * tp + 1) * NST + st, base + 64:base + 128] = 1.0
    w1s = np.ascontiguousarray(
        w1.transpose(1, 0, 2).reshape(2 * F, T * H)).astype(BF16)
    b1c = np.ascontiguousarray(
        b1.reshape(T, 2, 128).transpose(2, 0, 1).reshape(128, T * 2)
    ).astype(np.float32)
    w2s = np.ascontiguousarray(
        w2.reshape(T, 2, 128, O).transpose(2, 0, 1, 3).reshape(128, T * 2 * O)
    ).astype(BF16)
    b2pk = np.zeros((128, 2), np.float32)
    for tp in range(2):
        b2pk[0:64, tp] = b2[2 * tp]
        b2pk[64:128, tp] = b2[2 * tp + 1]
    ow1s = np.ascontiguousarray(ow1).astype(BF16)              # [128, H]
    ow1b0 = np.ascontiguousarray(ow1[N:2 * N]).astype(BF16)    # agg rows, base 0
    ob1c = np.ascontiguousarray(ob1.reshape(2, 128).T).astype(np.float32)
    ow2s = np.ascontiguousarray(
        ow2.reshape(2, 128, H).transpose(1, 0, 2).reshape(128, 2 * H)).astype(BF16)
    ob2c = np.ascontiguousarray(ob2.reshape(2, 128).T).astype(np.float32)
    ow3s = np.ascontiguousarray(
        ow3.reshape(2, 128, O).transpose(1, 0, 2).reshape(128, 2 * O)).astype(BF16)
    ob3c = np.ascontiguousarray(ob3.reshape(O, 1)).astype(np.float32)
    return dict(
        rsT=rsT.astype(BF16), rrT=rrT.astype(BF16), sel2=sel2.astype(BF16),
        w1s=w1s, b1c=b1c, w2s=w2s, b2p=b2pk,
        ow1s=ow1s, ow1b0=ow1b0, ob1c=ob1c, ow2s=ow2s, ob2c=ob2c,
        ow3s=ow3s, ob3c=ob3c,
    )


def prep_batch(x, rel_type, e_of_edge, rec_idx):
    """Per-core batched tensors: x, xT, rel_type as [T*NST, 512] rows."""
    bpc = x.shape[0]
    xT = np.ascontiguousarray(x.transpose(0, 2, 1))            # [bpc, F, N]
    rt_pad = np.zeros((bpc, EP, T), np.float32)
    rt_pad[:, e_of_edge, :] = rel_type                          # diag stays 0
    # row t*NST+st holds rel_type for pairs [st*512, (st+1)*512), type t
    rt32 = np.ascontiguousarray(
        rt_pad.reshape(bpc, NST, 512, T).transpose(0, 3, 1, 2).reshape(
            bpc, T * NST, 512)
    ).astype(BF16)
    return dict(x=np.ascontiguousarray(x).astype(BF16), xT=xT.astype(BF16),
                rt32=rt32)


def kernel(**inputs):
    from concourse.bass_utils import run_bass_kernel_spmd

    f32arrs = {k: np.asarray(v, dtype=np.float32) for k, v in inputs.items()}
    shared = prep_shared(
        f32arrs["w1"], f32arrs["b1"], f32arrs["w2"], f32arrs["b2"],
        f32arrs["ow1"], f32arrs["ob1"], f32arrs["ow2"], f32arrs["ob2"],
        f32arrs["ow3"], f32arrs["ob3"],
    )
    e_of_edge, rec_idx = edge_maps(f32arrs["rel_rec"], f32arrs["rel_send"])
    in_maps = []
    for c in range(NCORES):
        sl = slice(c * BPC, (c + 1) * BPC)
        m = dict(shared)
        m.update(prep_batch(
            f32arrs["x"][sl], f32arrs["rel_type"][sl], e_of_edge, rec_idx))
        in_maps.append(m)

    nc = build_nc(BPC)
    res = run_bass_kernel_spmd(nc, in_maps, list(range(NCORES)))
    # y per core: [O, BPC*N] -> [BPC, N, O]; concat -> full [B, N, O]
    y = np.concatenate(
        [res.results[c]["y"].reshape(O, BPC, N).transpose(1, 2, 0)
         for c in range(NCORES)], axis=0)
    return np.ascontiguousarray(y).astype(np.float32)


if __name__ == "__main__":
    # smoke: random inputs, shape check only
    rng = np.random.default_rng(0)
    eye = np.eye(N, dtype=np.float32)
    si, ri = [], []
    for i in range(N):
        for j in range(N):
            if i != j:
                si.append(i)
                ri.append(j)
    inputs = {
        "x": rng.standard_normal((B, N, F), dtype=np.float32),
        "rel_type": rng.random((B, E, T), dtype=np.float32),
        "rel_rec": eye[np.array(ri)],
        "rel_send": eye[np.array(si)],
        "w1": rng.standard_normal((T, 2 * F, H), dtype=np.float32) * 0.1,
        "b1": rng.standard_normal((T, H), dtype=np.float32) * 0.1,
        "w2": rng.standard_normal((T, H, O), dtype=np.float32) * 0.1,
        "b2": rng.standard_normal((T, O), dtype=np.float32) * 0.1,
        "ow1": rng.standard_normal((F + O, H), dtype=np.float32) * 0.1,
        "ob1": rng.standard_normal((H,), dtype=np.float32) * 0.1,
        "ow2": rng.standard_normal((H, H), dtype=np.float32) * 0.1,
        "ob2": rng.standard_normal((H,), dtype=np.float32) * 0.1,
        "ow3": rng.standard_normal((H, O), dtype=np.float32) * 0.1,
        "ob3": rng.standard_normal((O,), dtype=np.float32) * 0.1,
    }
    y = kernel(**inputs)
    print("y", y.shape, y.dtype)
